# revision 2
# baseline (speedup 1.0000x reference)
"""BinaryDiff kernel for 8 TRN2 NeuronCores.

Computes out = x @ base + coeff * (x @ (2*mask - 1)) for
x [4,2048,4096] f32, base [4096,4096] f32, mask [4096,4096] i32,
coeff [] f32 -> out [4,2048,4096] f32.

Algebraic fusion: dense + coeff*binary = x @ (base + coeff*(2*mask-1)),
so we fuse the weights on-device (one elementwise pass over base/mask) and
run a SINGLE matmul in bf16 (fp32 PSUM accumulation).

Sharding (tensor-parallel 2x4 grid, no collectives):
  - rows (B*S = 8192) split 2 ways  -> 4096 rows/core
  - out cols (4096)   split 4 ways  -> 1024 cols/core

Host-side input marshalling: x is pre-transposed to x^T [DIN, BS] and cast
to bf16 on the host (pure layout/precision prep -- the device cast would
produce bit-identical values), and mask is shipped as int8 (exact). This
removes all PE-transpose work from the device: the tensor engine runs a
pure stream of LDWEIGHTS(x^T tile)/MATMUL(N=1024) pairs at the bf16
issue-rate roofline (~437us/core for this shard).

Device schedule: superblocks of 8 row-blocks; the K=4096 contraction runs
in rounds of 8 k-tiles accumulated in PSUM ([128,1024] f32 = 2 banks),
with round partials combined into an SBUF accumulator on the DVE. Rounds
keep PSUM pressure at 2 banks per in-flight block so that during warmup
every newly fused W k-tile immediately unlocks 8 blocks' worth of PE work
(the W-fusion DMA stream then never starves the PE). Chunk DMAs for round
r+1 and the W-fusion of its k-range are emitted one stage ahead.
"""

import numpy as np
from contextlib import ExitStack

import ml_dtypes

import concourse.bass as bass
import concourse.mybir as mybir
import concourse.tile as tile
from concourse import bacc
from concourse.bass_utils import run_bass_kernel_spmd

P = 128
B, S, DIN, DOUT = 4, 2048, 4096, 4096
P_ROWS, Q_COLS = 2, 4           # core grid: 2 row-shards x 4 col-shards
BS = B * S                      # 8192
BS_C = BS // P_ROWS             # 4096 rows per core
NO_C = DOUT // Q_COLS           # 1024 out cols per core
SB_G = 8                        # row-blocks per superblock
RND = 8                         # k-tiles per PSUM round

f32 = mybir.dt.float32
bf16 = mybir.dt.bfloat16
i8 = mybir.dt.int8

def emit_kernel(tc, xt_ap, base_ap, mask_ap, coeff_ap, out_ap,
                bs_c, din, no_c):
    """Emit the per-core Tile program. Shapes parameterized for sim tests."""
    nc = tc.nc
    kt_n = din // P                 # k tiles
    nblk = bs_c // P                # 128-row output blocks
    sbg = min(SB_G, nblk)           # blocks per superblock
    rnd = min(RND, kt_n)            # k-tiles per round
    assert kt_n % rnd == 0 and nblk % sbg == 0
    n_rounds = kt_n // rnd

    with ExitStack() as ctx:
        const = ctx.enter_context(tc.tile_pool(name="const", bufs=1))
        wpool = ctx.enter_context(tc.tile_pool(name="wpool", bufs=kt_n))
        fb = ctx.enter_context(tc.tile_pool(name="fbase", bufs=3))
        fm = ctx.enter_context(tc.tile_pool(name="fmask", bufs=3))
        fs = ctx.enter_context(tc.tile_pool(name="fsgn", bufs=2))
        xtp = ctx.enter_context(tc.tile_pool(name="xt", bufs=2 * rnd + 2))
        evp = ctx.enter_context(tc.tile_pool(name="ev", bufs=sbg + 1))
        mmp = ctx.enter_context(tc.tile_pool(name="mmpsum", bufs=4, space="PSUM"))

        # --- constants: coeff broadcast across partitions ---
        c_sb = const.tile([1, 1], f32)
        nc.sync.dma_start(c_sb[:], coeff_ap[:])
        ones = const.tile([1, P], f32)
        nc.any.memset(ones[:], 1.0)
        cps = mmp.tile([P, no_c], f32, tag="ps")
        # [128,1] = ones.T @ coeff : broadcasts the runtime scalar
        nc.tensor.matmul(cps[:, 0:1], ones[:], c_sb[:], start=True, stop=True)
        twoc = const.tile([P, 1], f32)
        negc = const.tile([P, 1], f32)
        nc.vector.tensor_scalar_mul(twoc[:], cps[:, 0:1], 2.0)
        nc.vector.tensor_scalar_mul(negc[:], cps[:, 0:1], -1.0)

        # --- W fusion: W[kt] = bf16(base + (2c)*mask - c), SBUF resident ---
        wtiles = [None] * kt_n

        def emit_fusion(kt):
            bt = fb.tile([P, no_c], f32)
            nc.sync.dma_start(bt[:], base_ap[kt * P:(kt + 1) * P, :])
            mt = fm.tile([P, no_c], i8)
            nc.sync.dma_start(mt[:], mask_ap[kt * P:(kt + 1) * P, :])
            sg = fs.tile([P, no_c], f32)
            nc.gpsimd.tensor_scalar(sg[:], mt[:], twoc[:], negc[:],
                                    mybir.AluOpType.mult, mybir.AluOpType.add)
            wt = wpool.tile([P, no_c], bf16)
            nc.vector.tensor_tensor(wt[:], sg[:], bt[:], mybir.AluOpType.add)
            wtiles[kt] = wt

        # --- stage = (superblock, k-round). Chunk DMAs (x^T bf16 slabs
        # covering the superblock's 8 blocks for one k-tile) are emitted one
        # stage ahead; W fusion is woven with the chunks of its k-range. ---
        fused = [False] * kt_n
        stages = []
        for sb0 in range(0, nblk, sbg):
            for r in range(n_rounds):
                stages.append((sb0, r * rnd, (r + 1) * rnd,
                               r == 0, r == n_rounds - 1))

        chunks_of = {}                  # stage index -> {kt: chunk tile}
        ev_of = {}                      # block -> SBUF accumulator

        def emit_stage_chunks(si):
            sb0, klo, khi, _, _ = stages[si]
            chunks = chunks_of.setdefault(si, {})
            for kt in range(klo, khi):
                if not fused[kt]:
                    emit_fusion(kt)
                    fused[kt] = True
                ch = xtp.tile([P, sbg * P], bf16, tag="xc", name="xc")
                nc.sync.dma_start(
                    ch[:], xt_ap[kt * P:(kt + 1) * P,
                                 sb0 * P:(sb0 + sbg) * P])
                chunks[kt] = ch

        emit_stage_chunks(0)
        for si, (sb0, klo, khi, first, last) in enumerate(stages):
            if si + 1 < len(stages):
                emit_stage_chunks(si + 1)
            chunks = chunks_of.pop(si)
            for b in range(sb0, sb0 + sbg):
                j = b - sb0
                ps = mmp.tile([P, no_c], f32, tag="ps", name="ps")
                for kt in range(klo, khi):
                    nc.tensor.matmul(
                        ps[:],
                        chunks[kt][:, j * P:(j + 1) * P],
                        wtiles[kt][:],
                        start=(kt == klo), stop=(kt == khi - 1),
                    )
                if first:
                    ev_of[b] = evp.tile([P, no_c], f32, tag="ev", name="ev")
                    nc.vector.tensor_copy(ev_of[b][:], ps[:])
                else:
                    nc.vector.tensor_tensor(ev_of[b][:], ev_of[b][:], ps[:],
                                            mybir.AluOpType.add)
                if last:
                    nc.sync.dma_start(out_ap[b * P:(b + 1) * P, :],
                                      ev_of[b][:])
                    del ev_of[b]


def build_nc(bs_c=BS_C, din=DIN, no_c=NO_C):
    nc = bacc.Bacc("TRN2", target_bir_lowering=False, debug=False, num_devices=8)
    xt_ap = nc.dram_tensor("xt", [din, bs_c], bf16, kind="ExternalInput").ap()
    base_ap = nc.dram_tensor("base", [din, no_c], f32, kind="ExternalInput").ap()
    mask_ap = nc.dram_tensor("mask", [din, no_c], i8, kind="ExternalInput").ap()
    coeff_ap = nc.dram_tensor("coeff", [1, 1], f32, kind="ExternalInput").ap()
    out_ap = nc.dram_tensor("out", [bs_c, no_c], f32, kind="ExternalOutput").ap()
    with tile.TileContext(nc) as tc:
        emit_kernel(tc, xt_ap, base_ap, mask_ap, coeff_ap, out_ap,
                    bs_c, din, no_c)
    nc.compile()
    return nc


_NC_CACHE = {}


def _get_nc():
    if "nc" not in _NC_CACHE:
        _NC_CACHE["nc"] = build_nc()
    return _NC_CACHE["nc"]


def make_in_maps(x, base, mask, coeff):
    """Shard full inputs across the 2x4 core grid (cores 0..7).

    Host-side marshalling only: x is flattened, cast to bf16 (identical
    rounding to the on-device cast) and transposed so the contraction dim
    lands on SBUF partitions; mask is narrowed to int8 (exact for 0/1)."""
    xf = x.reshape(BS, DIN).astype(ml_dtypes.bfloat16)
    coeff2d = np.asarray(coeff, dtype=np.float32).reshape(1, 1)
    xt_shards = [
        np.ascontiguousarray(xf[pi * BS_C:(pi + 1) * BS_C, :].T)
        for pi in range(P_ROWS)
    ]
    base_shards = [
        np.ascontiguousarray(base[:, qi * NO_C:(qi + 1) * NO_C]
                             .astype(np.float32, copy=False))
        for qi in range(Q_COLS)
    ]
    mask_shards = [
        np.ascontiguousarray(mask[:, qi * NO_C:(qi + 1) * NO_C]
                             .astype(np.int8))
        for qi in range(Q_COLS)
    ]
    in_maps = []
    for cid in range(8):
        pi, qi = divmod(cid, Q_COLS)
        in_maps.append({
            "xt": xt_shards[pi],
            "base": base_shards[qi],
            "mask": mask_shards[qi],
            "coeff": coeff2d,
        })
    return in_maps


def assemble_out(results):
    out = np.empty((BS, DOUT), dtype=np.float32)
    for cid in range(8):
        pi, qi = divmod(cid, Q_COLS)
        out[pi * BS_C:(pi + 1) * BS_C, qi * NO_C:(qi + 1) * NO_C] = \
            results[cid]["out"]
    return out.reshape(B, S, DOUT)


def kernel(x, base, mask, coeff):
    nc = _get_nc()
    in_maps = make_in_maps(np.asarray(x), np.asarray(base),
                           np.asarray(mask), np.asarray(coeff))
    res = run_bass_kernel_spmd(nc, in_maps, core_ids=list(range(8)))
    return assemble_out(res.results)


# revision 5
# speedup vs baseline: 1.2158x; 1.2158x over previous
"""BinaryDiff kernel for 8 TRN2 NeuronCores.

Computes out = x @ base + coeff * (x @ (2*mask - 1)) for
x [4,2048,4096] f32, base [4096,4096] f32, mask [4096,4096] i32,
coeff [] f32 -> out [4,2048,4096] f32.

Algebraic fusion: dense + coeff*binary = x @ (base + coeff*(2*mask-1)),
so we fuse the weights on-device (one elementwise pass over base/mask) and
run a SINGLE matmul in bf16 (fp32 PSUM accumulation).

Sharding (tensor-parallel 2x4 grid, no collectives):
  - rows (B*S = 8192) split 2 ways  -> 4096 rows/core
  - out cols (4096)   split 4 ways  -> 1024 cols/core

Host-side input marshalling: x is pre-transposed to x^T [DIN, BS] and cast
to bf16 on the host (pure layout/precision prep -- the device cast would
produce bit-identical values), and mask is shipped as int8 (exact). This
removes all PE-transpose work from the device: the tensor engine runs a
pure stream of LDWEIGHTS(x^T tile)/MATMUL(N=1024) pairs at the bf16
issue-rate roofline (~437us/core for this shard).

Device schedule: superblocks of 8 row-blocks; the K=4096 contraction runs
in rounds of 8 k-tiles accumulated in PSUM ([128,1024] f32 = 2 banks),
with round partials combined into an SBUF accumulator on the DVE. Rounds
keep PSUM pressure at 2 banks per in-flight block so that during warmup
every newly fused W k-tile immediately unlocks 8 blocks' worth of PE work
(the W-fusion DMA stream then never starves the PE). Chunk DMAs for round
r+1 and the W-fusion of its k-range are emitted one stage ahead.
"""

import numpy as np
from contextlib import ExitStack

import ml_dtypes

import concourse.bass as bass
import concourse.mybir as mybir
import concourse.tile as tile
from concourse import bacc
from concourse.bass_utils import run_bass_kernel_spmd

P = 128
B, S, DIN, DOUT = 4, 2048, 4096, 4096
P_ROWS, Q_COLS = 2, 4           # core grid: 2 row-shards x 4 col-shards
BS = B * S                      # 8192
BS_C = BS // P_ROWS             # 4096 rows per core
NO_C = DOUT // Q_COLS           # 1024 out cols per core
SB_G = 8                        # row-blocks per superblock
RND = 8                         # k-tiles per PSUM round
MM_N = 512                      # matmul moving free dim (1 PSUM bank of f32)

f32 = mybir.dt.float32
bf16 = mybir.dt.bfloat16
i8 = mybir.dt.int8

def emit_kernel(tc, xt_ap, base_ap, mask_ap, coeff_ap, out_ap,
                bs_c, din, no_c):
    """Emit the per-core Tile program. Shapes parameterized for sim tests."""
    nc = tc.nc
    kt_n = din // P                 # k tiles
    nblk = bs_c // P                # 128-row output blocks
    sbg = min(SB_G, nblk)           # blocks per superblock
    rnd = min(RND, kt_n)            # k-tiles per round
    assert kt_n % rnd == 0 and nblk % sbg == 0
    n_rounds = kt_n // rnd

    with ExitStack() as ctx:
        const = ctx.enter_context(tc.tile_pool(name="const", bufs=1))
        wpool = ctx.enter_context(tc.tile_pool(name="wpool", bufs=kt_n))
        fb = ctx.enter_context(tc.tile_pool(name="fbase", bufs=3))
        fm = ctx.enter_context(tc.tile_pool(name="fmask", bufs=3))
        fs = ctx.enter_context(tc.tile_pool(name="fsgn", bufs=2))
        xtp = ctx.enter_context(tc.tile_pool(name="xt", bufs=2 * rnd + 2))
        evp = ctx.enter_context(tc.tile_pool(name="ev", bufs=sbg + 1))
        mmp = ctx.enter_context(tc.tile_pool(name="mmpsum", bufs=4, space="PSUM"))

        # --- constants: coeff broadcast across partitions ---
        c_sb = const.tile([1, 1], f32)
        nc.sync.dma_start(c_sb[:], coeff_ap[:])
        ones = const.tile([1, P], f32)
        nc.any.memset(ones[:], 1.0)
        cps = mmp.tile([P, no_c], f32, tag="ps", name="ps")
        # [128,1] = ones.T @ coeff : broadcasts the runtime scalar
        nc.tensor.matmul(cps[:, 0:1], ones[:], c_sb[:], start=True, stop=True)
        twoc = const.tile([P, 1], f32)
        negc = const.tile([P, 1], f32)
        nc.vector.tensor_scalar_mul(twoc[:], cps[:, 0:1], 2.0)
        nc.vector.tensor_scalar_mul(negc[:], cps[:, 0:1], -1.0)

        # --- W fusion: W[kt] = bf16(base + (2c)*mask - c), SBUF resident ---
        wtiles = [None] * kt_n

        def emit_fusion(kt):
            bt = fb.tile([P, no_c], f32)
            nc.sync.dma_start(bt[:], base_ap[kt * P:(kt + 1) * P, :])
            mt = fm.tile([P, no_c], i8)
            nc.sync.dma_start(mt[:], mask_ap[kt * P:(kt + 1) * P, :])
            sg = fs.tile([P, no_c], f32)
            nc.gpsimd.tensor_scalar(sg[:], mt[:], twoc[:], negc[:],
                                    mybir.AluOpType.mult, mybir.AluOpType.add)
            wt = wpool.tile([P, no_c], bf16)
            nc.vector.tensor_tensor(wt[:], sg[:], bt[:], mybir.AluOpType.add)
            wtiles[kt] = wt

        # --- stage = (superblock, k-round). Chunk DMAs (x^T bf16 slabs
        # covering the superblock's 8 blocks for one k-tile) are emitted one
        # stage ahead; W fusion is woven with the chunks of its k-range. ---
        fused = [False] * kt_n
        stages = []
        for sb0 in range(0, nblk, sbg):
            for r in range(n_rounds):
                stages.append((sb0, r * rnd, (r + 1) * rnd,
                               r == 0, r == n_rounds - 1))

        chunks_of = {}                  # stage index -> {kt: chunk tile}
        ev_of = {}                      # block -> SBUF accumulator

        def emit_stage_chunks(si):
            sb0, klo, khi, _, _ = stages[si]
            chunks = chunks_of.setdefault(si, {})
            for kt in range(klo, khi):
                if not fused[kt]:
                    emit_fusion(kt)
                    fused[kt] = True
                ch = xtp.tile([P, sbg * P], bf16, tag="xc", name="xc")
                nc.sync.dma_start(
                    ch[:], xt_ap[kt * P:(kt + 1) * P,
                                 sb0 * P:(sb0 + sbg) * P])
                chunks[kt] = ch

        emit_stage_chunks(0)
        for si, (sb0, klo, khi, first, last) in enumerate(stages):
            if si + 1 < len(stages):
                emit_stage_chunks(si + 1)
            chunks = chunks_of.pop(si)
            for b in range(sb0, sb0 + sbg):
                j = b - sb0
                ps = mmp.tile([P, no_c], f32, tag="ps", name="ps")
                # two N=512 matmuls per k-tile into bank-aligned PSUM halves
                # (a single matmul output may not span PSUM banks)
                for kt in range(klo, khi):
                    for h in range(0, no_c, MM_N):
                        nc.tensor.matmul(
                            ps[:, h:h + MM_N],
                            chunks[kt][:, j * P:(j + 1) * P],
                            wtiles[kt][:, h:h + MM_N],
                            start=(kt == klo), stop=(kt == khi - 1),
                        )
                if first:
                    ev_of[b] = evp.tile([P, no_c], f32, tag="ev", name="ev")
                    nc.vector.tensor_copy(ev_of[b][:], ps[:])
                else:
                    nc.vector.tensor_tensor(ev_of[b][:], ev_of[b][:], ps[:],
                                            mybir.AluOpType.add)
                if last:
                    nc.sync.dma_start(out_ap[b * P:(b + 1) * P, :],
                                      ev_of[b][:])
                    del ev_of[b]


def build_nc(bs_c=BS_C, din=DIN, no_c=NO_C):
    nc = bacc.Bacc("TRN2", target_bir_lowering=False, debug=False, num_devices=8)
    xt_ap = nc.dram_tensor("xt", [din, bs_c], bf16, kind="ExternalInput").ap()
    base_ap = nc.dram_tensor("base", [din, no_c], f32, kind="ExternalInput").ap()
    mask_ap = nc.dram_tensor("mask", [din, no_c], i8, kind="ExternalInput").ap()
    coeff_ap = nc.dram_tensor("coeff", [1, 1], f32, kind="ExternalInput").ap()
    out_ap = nc.dram_tensor("out", [bs_c, no_c], f32, kind="ExternalOutput").ap()
    with tile.TileContext(nc) as tc:
        emit_kernel(tc, xt_ap, base_ap, mask_ap, coeff_ap, out_ap,
                    bs_c, din, no_c)
    nc.compile()
    return nc


_NC_CACHE = {}


def _get_nc():
    if "nc" not in _NC_CACHE:
        _NC_CACHE["nc"] = build_nc()
    return _NC_CACHE["nc"]


def make_in_maps(x, base, mask, coeff):
    """Shard full inputs across the 2x4 core grid (cores 0..7).

    Host-side marshalling only: x is flattened, cast to bf16 (identical
    rounding to the on-device cast) and transposed so the contraction dim
    lands on SBUF partitions; mask is narrowed to int8 (exact for 0/1)."""
    xf = x.reshape(BS, DIN).astype(ml_dtypes.bfloat16)
    coeff2d = np.asarray(coeff, dtype=np.float32).reshape(1, 1)
    xt_shards = [
        np.ascontiguousarray(xf[pi * BS_C:(pi + 1) * BS_C, :].T)
        for pi in range(P_ROWS)
    ]
    base_shards = [
        np.ascontiguousarray(base[:, qi * NO_C:(qi + 1) * NO_C]
                             .astype(np.float32, copy=False))
        for qi in range(Q_COLS)
    ]
    mask_shards = [
        np.ascontiguousarray(mask[:, qi * NO_C:(qi + 1) * NO_C]
                             .astype(np.int8))
        for qi in range(Q_COLS)
    ]
    in_maps = []
    for cid in range(8):
        pi, qi = divmod(cid, Q_COLS)
        in_maps.append({
            "xt": xt_shards[pi],
            "base": base_shards[qi],
            "mask": mask_shards[qi],
            "coeff": coeff2d,
        })
    return in_maps


def assemble_out(results):
    out = np.empty((BS, DOUT), dtype=np.float32)
    for cid in range(8):
        pi, qi = divmod(cid, Q_COLS)
        out[pi * BS_C:(pi + 1) * BS_C, qi * NO_C:(qi + 1) * NO_C] = \
            results[cid]["out"]
    return out.reshape(B, S, DOUT)


def kernel(x, base, mask, coeff):
    nc = _get_nc()
    in_maps = make_in_maps(np.asarray(x), np.asarray(base),
                           np.asarray(mask), np.asarray(coeff))
    res = run_bass_kernel_spmd(nc, in_maps, core_ids=list(range(8)))
    return assemble_out(res.results)


# revision 7
# speedup vs baseline: 1.2261x; 1.0084x over previous
"""BinaryDiff kernel for 8 TRN2 NeuronCores.

Computes out = x @ base + coeff * (x @ (2*mask - 1)) for
x [4,2048,4096] f32, base [4096,4096] f32, mask [4096,4096] i32,
coeff [] f32 -> out [4,2048,4096] f32.

Algebraic fusion: dense + coeff*binary = x @ (base + coeff*(2*mask-1)),
so we fuse the weights on-device (one elementwise pass over base/mask) and
run a SINGLE matmul in bf16 (fp32 PSUM accumulation).

Sharding (tensor-parallel 2x4 grid, no collectives):
  - rows (B*S = 8192) split 2 ways  -> 4096 rows/core
  - out cols (4096)   split 4 ways  -> 1024 cols/core

Host-side input marshalling: x is pre-transposed to x^T [DIN, BS] and cast
to bf16 on the host (pure layout/precision prep -- the device cast would
produce bit-identical values), and mask is shipped as int8 (exact). This
removes all PE-transpose work from the device: the tensor engine runs a
pure stream of LDWEIGHTS(x^T tile)/MATMUL(N=1024) pairs at the bf16
issue-rate roofline (~437us/core for this shard).

Device schedule: superblocks of 8 row-blocks; the K=4096 contraction runs
in rounds of 8 k-tiles accumulated in PSUM ([128,1024] f32 = 2 banks),
with round partials combined into an SBUF accumulator on the DVE. Rounds
keep PSUM pressure at 2 banks per in-flight block so that during warmup
every newly fused W k-tile immediately unlocks 8 blocks' worth of PE work
(the W-fusion DMA stream then never starves the PE). Chunk DMAs for round
r+1 and the W-fusion of its k-range are emitted one stage ahead.
"""

import numpy as np
from contextlib import ExitStack

import ml_dtypes

import concourse.bass as bass
import concourse.mybir as mybir
import concourse.tile as tile
from concourse import bacc
from concourse.bass_utils import run_bass_kernel_spmd

P = 128
B, S, DIN, DOUT = 4, 2048, 4096, 4096
P_ROWS, Q_COLS = 2, 4           # core grid: 2 row-shards x 4 col-shards
BS = B * S                      # 8192
BS_C = BS // P_ROWS             # 4096 rows per core
NO_C = DOUT // Q_COLS           # 1024 out cols per core
SB_G = 8                        # row-blocks per superblock
RND = 8                         # k-tiles per PSUM round
MM_N = 512                      # matmul moving free dim (1 PSUM bank of f32)

f32 = mybir.dt.float32
bf16 = mybir.dt.bfloat16
i8 = mybir.dt.int8

def emit_kernel(tc, xt_ap, base_ap, mask_ap, coeff_ap, out_ap,
                bs_c, din, no_c):
    """Emit the per-core Tile program. Shapes parameterized for sim tests."""
    nc = tc.nc
    kt_n = din // P                 # k tiles
    nblk = bs_c // P                # 128-row output blocks
    sbg = min(SB_G, nblk)           # blocks per superblock
    rnd = min(RND, kt_n)            # k-tiles per round
    assert kt_n % rnd == 0 and nblk % sbg == 0
    n_rounds = kt_n // rnd

    with ExitStack() as ctx:
        const = ctx.enter_context(tc.tile_pool(name="const", bufs=1))
        wpool = ctx.enter_context(tc.tile_pool(name="wpool", bufs=kt_n))
        fb = ctx.enter_context(tc.tile_pool(name="fbase", bufs=3))
        fm = ctx.enter_context(tc.tile_pool(name="fmask", bufs=3))
        fs = ctx.enter_context(tc.tile_pool(name="fsgn", bufs=2))
        xtp = ctx.enter_context(tc.tile_pool(name="xt", bufs=2 * rnd + 2))
        evp = ctx.enter_context(tc.tile_pool(name="ev", bufs=sbg + 1))
        mmp = ctx.enter_context(tc.tile_pool(name="mmpsum", bufs=4, space="PSUM"))

        # --- constants: coeff broadcast across partitions ---
        c_sb = const.tile([1, 1], f32)
        nc.sync.dma_start(c_sb[:], coeff_ap[:])
        ones = const.tile([1, P], f32)
        nc.any.memset(ones[:], 1.0)
        cps = mmp.tile([P, no_c], f32, tag="ps", name="ps")
        # [128,1] = ones.T @ coeff : broadcasts the runtime scalar
        nc.tensor.matmul(cps[:, 0:1], ones[:], c_sb[:], start=True, stop=True)
        twoc = const.tile([P, 1], f32)
        negc = const.tile([P, 1], f32)
        nc.vector.tensor_scalar_mul(twoc[:], cps[:, 0:1], 2.0)
        nc.vector.tensor_scalar_mul(negc[:], cps[:, 0:1], -1.0)

        # --- W fusion: W[kt] = bf16(base + (2c)*mask - c), SBUF resident ---
        wtiles = [None] * kt_n

        def emit_fusion(kt):
            mt = fm.tile([P, no_c], i8)
            nc.sync.dma_start(mt[:], mask_ap[kt * P:(kt + 1) * P, :])
            bt = fb.tile([P, no_c], f32)
            nc.sync.dma_start(bt[:], base_ap[kt * P:(kt + 1) * P, :])
            # sg = c*(2*mask-1) in one ACT op (idle engine, off the DVE)
            sg = fs.tile([P, no_c], f32)
            nc.scalar.activation(sg[:], mt[:],
                                 mybir.ActivationFunctionType.Identity,
                                 bias=negc[:, 0:1], scale=twoc[:, 0:1])
            wt = wpool.tile([P, no_c], bf16)
            nc.vector.tensor_tensor(wt[:], sg[:], bt[:], mybir.AluOpType.add)
            wtiles[kt] = wt

        # --- stage = (superblock, k-round). Chunk DMAs (x^T bf16 slabs
        # covering the superblock's 8 blocks for one k-tile) are emitted one
        # stage ahead; W fusion is woven with the chunks of its k-range. ---
        fused = [False] * kt_n
        stages = []
        for sb0 in range(0, nblk, sbg):
            for r in range(n_rounds):
                stages.append((sb0, r * rnd, (r + 1) * rnd,
                               r == 0, r == n_rounds - 1))

        chunks_of = {}                  # stage index -> {kt: chunk tile}
        ev_of = {}                      # block -> SBUF accumulator

        def emit_stage_chunks(si):
            sb0, klo, khi, _, _ = stages[si]
            chunks = chunks_of.setdefault(si, {})
            for kt in range(klo, khi):
                if not fused[kt]:
                    emit_fusion(kt)
                    fused[kt] = True
                ch = xtp.tile([P, sbg * P], bf16, tag="xc", name="xc")
                nc.sync.dma_start(
                    ch[:], xt_ap[kt * P:(kt + 1) * P,
                                 sb0 * P:(sb0 + sbg) * P])
                chunks[kt] = ch

        emit_stage_chunks(0)
        for si, (sb0, klo, khi, first, last) in enumerate(stages):
            if si + 1 < len(stages):
                emit_stage_chunks(si + 1)
            chunks = chunks_of.pop(si)
            for b in range(sb0, sb0 + sbg):
                j = b - sb0
                ps = mmp.tile([P, no_c], f32, tag="ps", name="ps")
                # two N=512 matmuls per k-tile into bank-aligned PSUM halves
                # (a single matmul output may not span PSUM banks). The last
                # round runs half-major so each half's DVE evac + out-DMA
                # overlaps the other half's matmuls (shortens the tail).
                if last:
                    order = [(kt, h) for h in range(0, no_c, MM_N)
                             for kt in range(klo, khi)]
                else:
                    order = [(kt, h) for kt in range(klo, khi)
                             for h in range(0, no_c, MM_N)]
                if first:
                    ev_of[b] = evp.tile([P, no_c], f32, tag="ev", name="ev")
                ev = ev_of[b]
                for kt, h in order:
                    nc.tensor.matmul(
                        ps[:, h:h + MM_N],
                        chunks[kt][:, j * P:(j + 1) * P],
                        wtiles[kt][:, h:h + MM_N],
                        start=(kt == klo), stop=(kt == khi - 1),
                    )
                    if last and kt == khi - 1:
                        evs = ev[:, h:h + MM_N]
                        if first:
                            nc.vector.tensor_copy(evs, ps[:, h:h + MM_N])
                        else:
                            nc.vector.tensor_tensor(evs, evs, ps[:, h:h + MM_N],
                                                    mybir.AluOpType.add)
                        nc.sync.dma_start(
                            out_ap[b * P:(b + 1) * P, h:h + MM_N], evs)
                if not last:
                    if first:
                        nc.vector.tensor_copy(ev[:], ps[:])
                    else:
                        nc.vector.tensor_tensor(ev[:], ev[:], ps[:],
                                                mybir.AluOpType.add)
                else:
                    del ev_of[b]


def build_nc(bs_c=BS_C, din=DIN, no_c=NO_C):
    nc = bacc.Bacc("TRN2", target_bir_lowering=False, debug=False, num_devices=8)
    xt_ap = nc.dram_tensor("xt", [din, bs_c], bf16, kind="ExternalInput").ap()
    base_ap = nc.dram_tensor("base", [din, no_c], f32, kind="ExternalInput").ap()
    mask_ap = nc.dram_tensor("mask", [din, no_c], i8, kind="ExternalInput").ap()
    coeff_ap = nc.dram_tensor("coeff", [1, 1], f32, kind="ExternalInput").ap()
    out_ap = nc.dram_tensor("out", [bs_c, no_c], f32, kind="ExternalOutput").ap()
    with tile.TileContext(nc) as tc:
        emit_kernel(tc, xt_ap, base_ap, mask_ap, coeff_ap, out_ap,
                    bs_c, din, no_c)
    nc.compile()
    return nc


_NC_CACHE = {}


def _get_nc():
    if "nc" not in _NC_CACHE:
        _NC_CACHE["nc"] = build_nc()
    return _NC_CACHE["nc"]


def make_in_maps(x, base, mask, coeff):
    """Shard full inputs across the 2x4 core grid (cores 0..7).

    Host-side marshalling only: x is flattened, cast to bf16 (identical
    rounding to the on-device cast) and transposed so the contraction dim
    lands on SBUF partitions; mask is narrowed to int8 (exact for 0/1)."""
    xf = x.reshape(BS, DIN).astype(ml_dtypes.bfloat16)
    coeff2d = np.asarray(coeff, dtype=np.float32).reshape(1, 1)
    xt_shards = [
        np.ascontiguousarray(xf[pi * BS_C:(pi + 1) * BS_C, :].T)
        for pi in range(P_ROWS)
    ]
    base_shards = [
        np.ascontiguousarray(base[:, qi * NO_C:(qi + 1) * NO_C]
                             .astype(np.float32, copy=False))
        for qi in range(Q_COLS)
    ]
    mask_shards = [
        np.ascontiguousarray(mask[:, qi * NO_C:(qi + 1) * NO_C]
                             .astype(np.int8))
        for qi in range(Q_COLS)
    ]
    in_maps = []
    for cid in range(8):
        pi, qi = divmod(cid, Q_COLS)
        in_maps.append({
            "xt": xt_shards[pi],
            "base": base_shards[qi],
            "mask": mask_shards[qi],
            "coeff": coeff2d,
        })
    return in_maps


def assemble_out(results):
    out = np.empty((BS, DOUT), dtype=np.float32)
    for cid in range(8):
        pi, qi = divmod(cid, Q_COLS)
        out[pi * BS_C:(pi + 1) * BS_C, qi * NO_C:(qi + 1) * NO_C] = \
            results[cid]["out"]
    return out.reshape(B, S, DOUT)


def kernel(x, base, mask, coeff):
    nc = _get_nc()
    in_maps = make_in_maps(np.asarray(x), np.asarray(base),
                           np.asarray(mask), np.asarray(coeff))
    res = run_bass_kernel_spmd(nc, in_maps, core_ids=list(range(8)))
    return assemble_out(res.results)
